# revision 1
# baseline (speedup 1.0000x reference)
"""Trainium2 Bass kernel for CRF Viterbi decode (nn_CRF).

Problem (hardcoded): x[64, 512, 1024] @ kernel[1024, 128] + bias -> logits
[B, T, U]; boundary energies added on first/last timestep; Viterbi decode
with transition matrix chain_kernel[128, 128]; returns tags as float32.

Strategy
--------
Data-parallel over 8 NeuronCores: 8 batch elements per core.

Device (per core):
  1. logits matmul: x^T (pre-transposed on host to [D, (t, b)] layout)
     against kernel tiles, accumulated in PSUM over K=1024, evacuated by
     the scalar engine (bias fused) into an SBUF-resident logitsT[u, (t,b)].
  2. Viterbi forward scan (511 serial steps), batches split into 2 groups
     of 4 that pipeline across engines. Per step and group:
       - PE broadcasts v rows into PSUM (K=1 matmuls against a ones row)
         and accumulates trans[i, j] on top via an identity-tiled matmul
         (exact fp32: each PSUM element sees v[b,i] + trans[i,j] as a
         single fp32 add).
       - DVE does a segmented reduce-max over i -> max values [u, 4].
       - DVE adds logit_t -> new v, written into an SBUF vhist tile.
       - PE transposes the new v column block; ACT evacuates it to SBUF
         as rows for the next step's broadcast matmuls.
     Only max VALUES are kept (no argmax on device) - backpointers are
     reconstructed exactly on the host from vhist, since fp32 add/max
     here are bit-exact reproducible.
  3. vhist is DMAd out chunk-by-chunk as the scan progresses.

Host: shard/pre-transpose inputs, run SPMD on cores 0-7, then backtrace:
  tag_t = argmax_i(v_t[b, i] + trans[i, tag_{t+1}]) - bit-identical to the
  backpointers the device forward pass implies.
"""

import os

import numpy as np

import concourse.bass as bass
import concourse.mybir as mybir
from concourse.tile import TileContext
from concourse.bass_utils import run_bass_kernel_spmd

F32 = mybir.dt.float32

# Problem constants
B, T, D, U = 64, 512, 1024, 128
NCORES = 8
BL = B // NCORES           # batches per core (8)
NG = 2                     # batch pipeline groups
GB = BL // NG              # batches per group (4)

last_results = None        # BassKernelResults of the most recent kernel() run


def split_multi_waits(nc):
    """The walrus build in this container encodes at most ONE sync wait per
    compute/DMA instruction ("Too many sync wait commands" otherwise). Hoist
    all but the last wait of any multi-wait instruction onto standalone
    same-engine EventSemaphore ops placed immediately before it (engine
    queues execute in order, so semantics are preserved)."""
    for f in nc.m.functions:
        for blk in f.blocks:
            new_insts = []
            changed = False
            for inst in blk.instructions:
                si = inst.sync_info
                if si is not None and len(si.on_wait) > 1:
                    waits = list(si.on_wait)
                    for k, w in enumerate(waits[:-1]):
                        new_insts.append(mybir.InstEventSemaphore(
                            name=f"{inst.name}-sw{k}",
                            engine=inst.engine,
                            ins=[], outs=[],
                            sync_info=mybir.SyncInfo(on_wait=[w], on_update=[]),
                        ))
                    inst.sync_info = mybir.SyncInfo(
                        on_wait=[waits[-1]], on_update=list(si.on_update))
                    changed = True
                new_insts.append(inst)
            if changed:
                blk.instructions = new_insts
    return nc


def build_program(t_steps=T, d_dim=D, split_waits=True, scan_reps=1,
                  loop_reps=None, warm=0):
    nt = t_steps * BL                       # columns in (t, b) layout
    ch = min(512, nt)                       # DMA/matmul chunk width
    nch = nt // ch
    kblocks = d_dim // 128

    nc = bass.Bass(trn_type="TRN2")

    xdt = nc.dram_tensor("xdt", [d_dim, nt], F32, kind="ExternalInput")
    ker = nc.dram_tensor("ker", [d_dim, U], F32, kind="ExternalInput")
    translhs = nc.dram_tensor("translhs", [U, U], F32, kind="ExternalInput")
    ident = nc.dram_tensor("ident", [U, U], F32, kind="ExternalInput")
    lbv = nc.dram_tensor("lbv", [U, 1], F32, kind="ExternalInput")
    rbv = nc.dram_tensor("rbv", [U, 1], F32, kind="ExternalInput")
    biasrow = nc.dram_tensor("biasrow", [1, U], F32, kind="ExternalInput")
    onesrow = nc.dram_tensor("onesrow", [1, 512], F32, kind="ExternalInput")
    vout = nc.dram_tensor("vout", [U, nt], F32, kind="ExternalOutput")

    with TileContext(nc) as tc:
        with (
            tc.tile_pool(name="const", bufs=1) as cpool,
            tc.tile_pool(name="xp", bufs=10) as xpool,
            tc.tile_pool(name="big", bufs=1) as bigpool,
            tc.tile_pool(name="mx", bufs=3) as mxpool,
            tc.tile_pool(name="mmps", bufs=2, space="PSUM") as mmpool,
            tc.tile_pool(name="scps", bufs=2, space="PSUM") as scpool,
            tc.tile_pool(name="wmps", bufs=1, space="PSUM") as wmpool,
        ):
            # ---- constants into SBUF ----
            ker_sb = []
            for kb in range(kblocks):
                kt = cpool.tile([128, U], F32, tag=f"ker{kb}")
                nc.sync.dma_start(out=kt[:, :], in_=ker[kb * 128:(kb + 1) * 128, :])
                ker_sb.append(kt)
            trans_sb = cpool.tile([U, U], F32, tag="trans")
            nc.sync.dma_start(out=trans_sb[:, :], in_=translhs[:, :])
            ident_sb = cpool.tile([U, U], F32, tag="ident")
            nc.sync.dma_start(out=ident_sb[:, :], in_=ident[:, :])
            lb_sb = cpool.tile([U, 1], F32, tag="lb")
            nc.sync.dma_start(out=lb_sb[:, :], in_=lbv[:, :])
            rb_sb = cpool.tile([U, 1], F32, tag="rb")
            nc.sync.dma_start(out=rb_sb[:, :], in_=rbv[:, :])
            biasrow_sb = cpool.tile([1, U], F32, tag="biasrow")
            nc.sync.dma_start(out=biasrow_sb[:, :], in_=biasrow[:, :])
            onesrow_sb = cpool.tile([1, 512], F32, tag="onesrow")
            nc.sync.dma_start(out=onesrow_sb[:, :], in_=onesrow[:, :])

            logitsT = bigpool.tile([U, nt], F32, tag="logitsT")
            # per-group v history (decoupled so the two batch-group
            # pipelines never serialize on tile dependency tracking);
            # group g columns: t * GB + bb
            vh = [bigpool.tile([U, nt // NG], F32, tag=f"vh{g}",
                                name=f"vh{g}")
                  for g in range(NG)]

            # ---- phase 1: logits = kernel.T @ x (+bias) ----
            for c in range(nch):
                ps = mmpool.tile([128, ch], F32, tag="mm")
                for kb in range(kblocks):
                    xt = xpool.tile([128, ch], F32, tag="x")
                    nc.sync.dma_start(
                        out=xt[:, :],
                        in_=xdt[kb * 128:(kb + 1) * 128, c * ch:(c + 1) * ch],
                    )
                    nc.tensor.matmul(
                        out=ps[:, :], lhsT=ker_sb[kb][:, :], rhs=xt[:, :],
                        start=(kb == 0), stop=False,
                    )
                nc.tensor.matmul(
                    out=ps[:, :], lhsT=biasrow_sb[0:1, :],
                    rhs=onesrow_sb[0:1, 0:ch], start=False, stop=True,
                )
                nc.scalar.copy(
                    out=logitsT[:, c * ch:(c + 1) * ch], in_=ps[:, :],
                )

            # right boundary folded into the last timestep's logits
            nc.vector.tensor_scalar_add(
                out=logitsT[:, (t_steps - 1) * BL:],
                in0=logitsT[:, (t_steps - 1) * BL:],
                scalar1=rb_sb[:, 0:1],
            )
            # ---- phase 2: Viterbi forward scan ----
            # v broadcast across partitions: matmul with the previous v
            # column as stationary operand, broadcast along its free dim
            # (step-0 AP), against an identity moving operand:
            #   out[p, i] = sum_k v[k] * I[k, i] = v[i]  for every p.
            # scan_reps > 1 repeats the whole scan (for differential
            # wall-clock timing); results are identical each rep.
            steps_per_chunk = ch // BL
            gch = steps_per_chunk * GB          # per-group chunk width
            import contextlib
            rep_ctx = (tc.For_i(0, loop_reps, 1) if loop_reps
                       else contextlib.nullcontext())
            with rep_ctx:
             for _rep in range(scan_reps):
              # v_0 = logits_0 + left boundary
              for g in range(NG):
                nc.vector.tensor_scalar_add(
                    out=vh[g][:, 0:GB], in0=logitsT[:, g * GB:(g + 1) * GB],
                    scalar1=lb_sb[:, 0:1],
                )
              for t in range(1, t_steps):
                  for g in range(NG):
                      lcols0 = t * BL + g * GB    # logitsT columns
                      vcols0 = t * GB             # vh[g] columns
                      pcol0 = (t - 1) * GB
                      sc = scpool.tile([128, GB * U], F32, tag=f"sc{g}")
                      for bb in range(GB):
                          vcol = vh[g][:, pcol0 + bb:pcol0 + bb + 1]
                          nc.tensor.matmul(
                              out=sc[:, bb * U:(bb + 1) * U],
                              lhsT=vcol.broadcast_to([U, U]), rhs=ident_sb[:, :],
                              start=(bb == 0), stop=False, skip_group_check=True,
                              is_transpose=True,
                          )
                      for bb in range(GB):
                          nc.tensor.matmul(
                              out=sc[:, bb * U:(bb + 1) * U],
                              lhsT=trans_sb[:, :], rhs=ident_sb[:, :],
                              start=False, stop=(bb == GB - 1),
                              skip_group_check=True, is_transpose=True,
                          )
                      # optional HAM-warming filler: keeps the PE p-state
                      # hot across the per-step stall waiting for v(t)
                      for _w in range(warm):
                          wt = wmpool.tile([U, U], F32, tag="warm")
                          nc.tensor.matmul(
                              out=wt[:, :], lhsT=ident_sb[:, :],
                              rhs=ident_sb[:, :], start=True, stop=True,
                              skip_group_check=True, is_transpose=True,
                          )
                      mx = mxpool.tile([U, GB], F32, tag=f"mx{g}")
                      nc.vector.tensor_reduce(
                          out=mx[:, :],
                          in_=sc[:, :].rearrange("p (b i) -> p b i", i=U),
                          axis=mybir.AxisListType.X, op=mybir.AluOpType.max,
                      )
                      # logit add on the otherwise-idle ACT engine, one
                      # column at a time (bias = per-partition logitsT col);
                      # lets each next-step broadcast start as soon as its
                      # own column is written
                      for bb in range(GB):
                          nc.scalar.activation(
                              out=vh[g][:, vcols0 + bb:vcols0 + bb + 1],
                              in_=mx[:, bb:bb + 1],
                              func=mybir.ActivationFunctionType.Identity,
                              bias=logitsT[:, lcols0 + bb:lcols0 + bb + 1],
                          )
                  if (t + 1) % steps_per_chunk == 0:
                      c = (t + 1) // steps_per_chunk - 1
                      for g in range(NG):
                          nc.sync.dma_start(
                              out=vout[:, g * (nt // NG) + c * gch:
                                       g * (nt // NG) + (c + 1) * gch],
                              in_=vh[g][:, c * gch:(c + 1) * gch],
                          )
    return split_multi_waits(nc) if split_waits else nc


def make_in_map(x_core, ker, bias, trans, lb, rb, t_steps=T, d_dim=D):
    """x_core: [BL, t_steps, d_dim] float32."""
    nt = t_steps * BL
    xdt = np.ascontiguousarray(x_core.transpose(2, 1, 0)).reshape(d_dim, nt)
    return {
        "xdt": xdt.astype(np.float32),
        "ker": np.ascontiguousarray(ker, dtype=np.float32),
        "biasrow": np.ascontiguousarray(bias, dtype=np.float32).reshape(1, U),
        "onesrow": np.ones((1, 512), dtype=np.float32),
        "translhs": np.ascontiguousarray(trans, dtype=np.float32),
        "ident": np.eye(U, dtype=np.float32),
        "lbv": np.ascontiguousarray(lb, dtype=np.float32).reshape(U, 1),
        "rbv": np.ascontiguousarray(rb, dtype=np.float32).reshape(U, 1),
    }


def backtrace(v, trans):
    """v: [b, t, u] forward max values; trans: [u, u]. Returns int tags [b, t]."""
    nb, nt, nu = v.shape
    tags = np.zeros((nb, nt), dtype=np.int64)
    cur = np.argmax(v[:, -1, :], axis=1)
    tags[:, -1] = cur
    for t in range(nt - 2, -1, -1):
        scores = v[:, t, :] + trans[:, cur].T     # fp32, same as device order
        cur = np.argmax(scores, axis=1)
        tags[:, t] = cur
    return tags


def vout_to_v(vout_core, t_steps=T):
    """vout [U, (g, t, bb)] -> v [BL, t, U] with b = g * GB + bb."""
    v = vout_core.reshape(U, NG, t_steps, GB)     # [u, g, t, bb]
    return np.ascontiguousarray(v.transpose(1, 3, 2, 0).reshape(BL, t_steps, U))


def kernel(x, kernel, bias, chain_kernel, left_boundary, right_boundary):
    x = np.asarray(x, dtype=np.float32)
    ker = np.asarray(kernel, dtype=np.float32)
    bias = np.asarray(bias, dtype=np.float32)
    trans = np.asarray(chain_kernel, dtype=np.float32)
    lb = np.asarray(left_boundary, dtype=np.float32)
    rb = np.asarray(right_boundary, dtype=np.float32)

    nc = build_program()
    in_maps = [
        make_in_map(x[c * BL:(c + 1) * BL], ker, bias, trans, lb, rb)
        for c in range(NCORES)
    ]
    kwargs = {}
    if os.environ.get("CRF_TRACE"):
        kwargs = {"trace": True, "tmpdir": os.environ.get("CRF_TRACE_DIR") or None}
    res = run_bass_kernel_spmd(nc, in_maps, core_ids=list(range(NCORES)), **kwargs)
    global last_results
    last_results = res
    v = np.concatenate(
        [vout_to_v(np.asarray(r["vout"])) for r in res.results], axis=0)
    tags = backtrace(v, trans)
    return tags.astype(np.float32)

